# revision 1
# baseline (speedup 1.0000x reference)
"""TRN2 Bass kernel for nn_FFLayer: y = relu(l2norm_rows(x) @ W.T + b).

Strategy: data-parallel over the batch dim across 8 NeuronCores.
Each core gets a 1024-row shard of x (shipped pre-transposed as x^T so the
contraction dim lands on SBUF partitions), the full W (shipped as W^T), and b.

Per-core kernel (all matmuls in float32r: full fp32 I/O, ~1.2e-4 rel-err,
4x the fp32 matmul rate):
  - norms: square x^T tiles on DVE (bf16 scratch), reduce over k with a
    ones-vector matmul on the PE into PSUM -> sqrt(+eps) on ACT.
  - GEMM: out[m, n-chunk] accumulated over 32 k-tiles; bias is folded in as a
    rank-1 matmul (norm+eps) x b so scale+bias+relu collapses into the single
    ACT drain pass: y = Relu(psum * s) with per-partition scale s = 1/(norm+eps).
"""
import sys

sys.path.insert(0, "/opt/trn_rl_repo")

import numpy as np

import concourse.bacc as bacc
import concourse.bass as bass
import concourse.mybir as mybir
import concourse.tile as tile
from concourse.bass_utils import run_bass_kernel_spmd

F32 = mybir.dt.float32
F32R = mybir.dt.float32r
BF16 = mybir.dt.bfloat16
ACTF = mybir.ActivationFunctionType

N_CORES = 8
B, IN, OUT = 8192, 4096, 4096
MS = B // N_CORES          # 1024 rows per core
MT = MS // 128             # 8 m-tiles
KT = IN // 128             # 32 k-tiles
KQ = KT // 4               # k-tiles per W quarter-load
NCH = 256                  # n-chunk (moving cols per matmul; >=256 for f32r rate)
NC_N = OUT // NCH          # 16 chunks
EPS = 1e-4

_cached_nc = {}


def _build(reps=1):
    nc = bacc.Bacc("TRN2", target_bir_lowering=False, debug=False)

    # xs[mb*128 + p, kt*128 + m] = x_shard[mb*128 + m, kt*128 + p]
    # (per-m-block loads are fully contiguous)
    xs_d = nc.dram_tensor("xs", [MS, IN], F32R, kind="ExternalInput")
    # wt[c, q, p, j, n] = W[c*256 + n, (q*8 + j)*128 + p]
    # (per-quarter-chunk loads are fully contiguous)
    wt_d = nc.dram_tensor(
        "wt", [NC_N, 4, 128, KQ, NCH], F32R, kind="ExternalInput"
    )
    b_d = nc.dram_tensor("bias", [OUT], F32R, kind="ExternalInput")
    y_d = nc.dram_tensor("y", [MS, OUT], F32, kind="ExternalOutput")

    with tile.TileContext(nc) as tc:
        with (
            tc.tile_pool(name="xp", bufs=1) as xp,
            tc.tile_pool(name="wp", bufs=8) as wp,
            tc.tile_pool(name="sqp", bufs=1) as sqp,
            tc.tile_pool(name="op", bufs=2) as op,
            tc.tile_pool(name="rows", bufs=1) as rows,
            tc.tile_pool(name="bp", bufs=2) as bp,
            tc.tile_pool(name="npsum", bufs=2, space=bass.MemorySpace.PSUM) as npsum,
            tc.tile_pool(name="gpsum", bufs=6, space=bass.MemorySpace.PSUM) as gpsum,
        ):
            ones = rows.tile([128, 1], BF16, tag="ones")
            nrow_f = rows.tile([1, MS], F32, tag="nrow_f")    # norm + eps (fp32)
            nrow_r = rows.tile([1, MS], F32R, tag="nrow_r")   # norm + eps (f32r)
            ncol = rows.tile([128, MT], F32, tag="ncol")      # norm + eps, [m%128, mt]
            s_col = rows.tile([128, MT], F32, tag="s_col")    # 1/(norm+eps)

            nc.gpsimd.memset(ones[:], 1.0)

            # x^T m-blocks: one tile per m-block so matmul deps are exact
            xts = [None] * MT

            def load_x_block(mt):
                m0 = mt * 128
                t = xp.tile([128, KT, 128], F32R, tag=f"xt{mt}")
                nc.sync.dma_start(
                    t[:],
                    xs_d.ap()[m0 : m0 + 128, :].rearrange("p (kt m) -> p kt m", m=128),
                )
                xts[mt] = t

            def load_w_quarter(c, q):
                w = wp.tile([128, KQ, NCH], F32R, tag="wc")
                nc.sync.dma_start(w[:], wt_d.ap()[c, q])
                return w

            def load_b_chunk(c):
                bt = bp.tile([1, NCH], F32R, tag="bc")
                nc.sync.dma_start(
                    bt[:],
                    b_d.ap()[c * NCH : (c + 1) * NCH].rearrange("(o n) -> o n", o=1),
                )
                return bt

            def norms_for(mt):
                m0 = mt * 128
                npt = npsum.tile([1, 128], F32, tag="np")
                for q in range(4):
                    kt0 = q * KQ
                    sq = sqp.tile([128, KQ, 128], BF16, tag="sq")
                    nc.vector.tensor_mul(
                        sq[:],
                        xts[mt][:, kt0 : kt0 + KQ, :].bitcast(F32),
                        xts[mt][:, kt0 : kt0 + KQ, :].bitcast(F32),
                    )
                    for j in range(KQ):
                        kt = kt0 + j
                        nc.tensor.matmul(
                            npt[:],
                            ones[:],
                            sq[:, j, :],
                            start=(kt == 0),
                            stop=(kt == KT - 1),
                        )
                # norm = sqrt(sumsq); then += eps in place
                nc.scalar.activation(nrow_f[0:1, m0 : m0 + 128], npt[:], ACTF.Sqrt)
                nc.scalar.activation(
                    nrow_f[0:1, m0 : m0 + 128],
                    nrow_f[0:1, m0 : m0 + 128],
                    ACTF.Copy,
                    bias=EPS,
                )
                # f32r copy for the rank-1 bias matmul (SWDGE cast-DMA)
                nc.gpsimd.dma_start(
                    nrow_r[0:1, m0 : m0 + 128], nrow_f[0:1, m0 : m0 + 128]
                )
                # [1,128] free-run -> [128,1] partition fan-out, then reciprocal
                nc.gpsimd.dma_start(ncol[:, mt : mt + 1], nrow_f[0:1, m0 : m0 + 128])
                nc.vector.reciprocal(s_col[:, mt : mt + 1], ncol[:, mt : mt + 1])

            def gemm_group(ps, wq, bt, mt):
                """32 k-matmuls + rank-1 bias into a [128, NCH] psum tile."""
                m0 = mt * 128
                for kt in range(KT):
                    nc.tensor.matmul(
                        ps[:],
                        xts[mt][:, kt, :],
                        wq[kt // KQ][:, kt % KQ, :],
                        start=(kt == 0),
                        stop=False,
                    )
                nc.tensor.matmul(
                    ps[:],
                    nrow_r[0:1, m0 : m0 + 128],
                    bt[:],
                    start=False,
                    stop=True,
                )

            def drain(ps, mt, ncol0):
                m0 = mt * 128
                o = op.tile([128, NCH], F32, tag="o")
                nc.scalar.activation(
                    o[:], ps[:], ACTF.Relu, scale=s_col[:, mt : mt + 1]
                )
                nc.sync.dma_start(
                    y_d.ap()[m0 : m0 + 128, ncol0 : ncol0 + NCH], o[:]
                )

            def one_pass():
                # ---- startup: interleave x m-block loads with W chunk 0 load
                b_cur = load_b_chunk(0)
                load_x_block(0)
                w_cur = [load_w_quarter(0, q) for q in range(2)]
                load_x_block(1)
                w_cur += [load_w_quarter(0, q) for q in range(2, 4)]
                for mt in range(2, MT):
                    load_x_block(mt)

                # ---- 16 chunk passes; prefetch next chunk's W right after the
                # first group of the current pass (its slots freed a pass ago)
                for c in range(NC_N):
                    w_nxt = b_nxt = None
                    for mt in range(MT):
                        if c == 0:
                            norms_for(mt)
                        ps = gpsum.tile([128, NCH], F32, tag="gps")
                        gemm_group(ps, w_cur, b_cur, mt)
                        drain(ps, mt, c * NCH)
                        if mt == 0 and c + 1 < NC_N:
                            b_nxt = load_b_chunk(c + 1)
                            w_nxt = [load_w_quarter(c + 1, q) for q in range(4)]
                    w_cur, b_cur = w_nxt, b_nxt

            for _rep in range(reps):
                one_pass()

    nc.compile()
    return nc


def _get_nc(reps=1):
    if reps not in _cached_nc:
        _cached_nc[reps] = _build(reps)
    return _cached_nc[reps]


def prep_inputs(x, W, b):
    x = np.asarray(x, dtype=np.float32)
    W = np.asarray(W, dtype=np.float32)
    b = np.asarray(b, dtype=np.float32)
    # x: [core, mb, m, kt, p] -> [core, mb, p, kt, m]
    xs_all = np.ascontiguousarray(
        x.reshape(N_CORES, MT, 128, KT, 128).transpose(0, 1, 4, 3, 2)
    ).reshape(N_CORES, MS, IN)
    # W: [c, n, q, j, p] -> [c, q, p, j, n]
    w4 = np.ascontiguousarray(
        W.reshape(NC_N, NCH, 4, KQ, 128).transpose(0, 2, 4, 3, 1)
    )
    return [{"xs": xs_all[i], "wt": w4, "bias": b} for i in range(N_CORES)]


def kernel(x: np.ndarray, W: np.ndarray, b: np.ndarray, **run_kwargs) -> np.ndarray:
    nc = _get_nc()
    in_maps = prep_inputs(x, W, b)

    res = run_bass_kernel_spmd(nc, in_maps, list(range(N_CORES)), **run_kwargs)
    out = np.concatenate([res.results[i]["y"] for i in range(N_CORES)], axis=0)
    if run_kwargs:
        kernel.last_result = res
    return out

